# revision 26
# baseline (speedup 1.0000x reference)
"""Binarized 3-layer MLP on 8 TRN2 NeuronCores (data-parallel over batch).

Computation (matching the reference):
    h1  = x @ sign(W1).T          x: [65536, 784] fp32, W1: [400, 784]
    h2  = sign(h1) @ sign(W2).T   W2: [200, 400]
    out = sign(h2) @ sign(W3).T   W3: [10, 200]

Strategy (v3):
  - Batch sharded 8192 rows/core; weights replicated.
  - Layer 1 precision: x split as hi = e4m3(x) plus lo = fp16(x - hi).
    The hi pass runs as fp8 DoubleRow matmuls (2 K-rows per PE cell,
    K=256 per matmul); the lo pass runs as fp16 matmuls. Combined
    representation error ~2^-15 relative -> rel err ~8e-3 via sign-flip
    propagation, vs the 2e-2 gate.
  - K tail (rows 768:784): hi(e4m3->fp16 exact) and lo stacked as one
    32-row fp16 tile, replicated at partition strips 0/32/64 so the
    three m-tiles' tail matmuls run concurrently in one window.
  - m4 (h1 features 384:400): col-packed, groups of 4 chunks run their
    m4 matmuls in distinct 32-column PE groups concurrently.
  - Layer 2 (K=400): two fp8 DoubleRow matmuls per m-tile. The second
    pairs features 256:384 (a1x pair 0) with the packed m4 signs
    (a1x pair 1 = copy of a13p); per-chunk-position weight tiles w2b[jj]
    zero out all but that chunk's 16-row strip.
  - Layer 3 (K=200): one fp8 DoubleRow matmul per chunk; the a2 pair
    tile holds features 0:128 at pair 0 and 128:200 at pair 1 with
    zeroed weights over the 72:128 garbage partitions.
  - Sign() outputs are exactly representable in e4m3, and PSUM
    accumulates fp32, so layers 2/3 are exact.
  - Issue schedule is software-pipelined: L2 lags L1 by one chunk and
    L3 by two, so Sign (scalar engine) latency is hidden under the next
    chunk's matmul streams, across group boundaries.
"""

import contextlib
import ctypes
import os
import sys
import types

import numpy as np
import ml_dtypes

import concourse.bacc as bacc
import concourse.mybir as mybir
import concourse.tile as tile
from concourse.bass_utils import run_bass_kernel_spmd


def _ensure_axon_hooks():
    """concourse's trace path imports antenv.axon_hooks, which this image
    lacks; register a ctypes-backed stand-in so trace=True (or a stray
    BASS_TRACE=1 in the environment) cannot crash the run."""
    try:
        import antenv.axon_hooks  # noqa: F401
        return
    except ImportError:
        pass

    so_path = "/opt/axon/libaxon_pjrt.so"
    hook = None
    if os.path.exists(so_path):
        try:
            lib = ctypes.CDLL(so_path)
            if hasattr(lib, "axon_start_nrt_profile"):
                lib.axon_start_nrt_profile.argtypes = [
                    ctypes.POINTER(ctypes.c_int64),
                    ctypes.c_size_t,
                ]
                lib.axon_start_nrt_profile.restype = ctypes.c_int64
                lib.axon_stop_nrt_profile.argtypes = [ctypes.c_char_p]
                lib.axon_stop_nrt_profile.restype = ctypes.c_int64

                @contextlib.contextmanager
                def _hook(output_dir, device_ids):
                    import jax

                    jax.devices()
                    if device_ids:
                        ids = (ctypes.c_int64 * len(device_ids))(*device_ids)
                        rc = lib.axon_start_nrt_profile(ids, len(device_ids))
                    else:
                        rc = lib.axon_start_nrt_profile(None, 0)
                    if rc != 0:
                        raise RuntimeError(f"axon_start_nrt_profile rc={rc}")
                    try:
                        yield
                    finally:
                        lib.axon_stop_nrt_profile(str(output_dir).encode())

                hook = _hook
        except OSError:
            pass

    mod = types.ModuleType("antenv.axon_hooks")
    mod.get_axon_ntff_profile_hook = lambda: hook
    mod.set_axon_ntff_profile_hook = lambda h: None
    sys.modules["antenv.axon_hooks"] = mod

    import concourse.bass_utils as _bu

    _bu.upload_artifacts = lambda tmpdir: tmpdir


E4M3 = np.dtype(ml_dtypes.float8_e4m3)

NCORES = 8
B = 65536
BL = B // NCORES          # 8192 rows per core
D0, H1, H2, DO = 784, 400, 200, 10
CH = 512                  # batch columns per chunk (PSUM bank = 512 fp32)
NCH = BL // CH            # 16 chunks per core
GRP = 4                   # chunks per m4 packing group
NG = NCH // GRP
KH = 3                    # hi-pass DoubleRow k-tiles (K=256 each, 768 rows)
KL = 6                    # lo-pass fp16 k-tiles (K=128 each, 768 rows)
NWARM = 120               # PE warmup matmuls (HAM ramp + DMA-wait cover)

_cache = {}


def _build():
    if "nc" in _cache:
        return _cache["nc"]

    f32 = mybir.dt.float32
    f16 = mybir.dt.float16
    f8 = mybir.dt.float8e4
    DR = mybir.MatmulPerfMode.DoubleRow
    Sign = mybir.ActivationFunctionType.Sign

    nc = bacc.Bacc("TRN2", debug=False, num_devices=NCORES)

    # x hi plane: [chunk][p, k, pair, col], K-row r = 256k + 128i + p
    d_xhi = nc.dram_tensor("xhi", [NCH, 128, KH, 2, CH], f8, kind="ExternalInput").ap()
    # x lo plane + tail: k 0:6 lo (r = 128k + p); k=6 = 32-row fp16 tail
    # (hi[768:784] ++ lo[768:784]) replicated at partition strips 0/32/64
    d_xlt = nc.dram_tensor("xlt", [NCH, 128, KL + 1, CH], f16, kind="ExternalInput").ap()
    d_w1h = nc.dram_tensor("w1h", [128, KH, 2, H1], f8, kind="ExternalInput").ap()
    d_w1l = nc.dram_tensor("w1l", [128, KL + 1, H1], f16, kind="ExternalInput").ap()
    d_w2a = nc.dram_tensor("w2a", [128, 2, 256], f8, kind="ExternalInput").ap()
    # w2b[jj]: pair 0 = W2 rows 256:384; pair 1 = rows 384:400 at partition
    # strip 32jj (zeros elsewhere, masking the other chunks' packed signs)
    d_w2b = nc.dram_tensor("w2b", [128, GRP, 2, 256], f8, kind="ExternalInput").ap()
    d_w3 = nc.dram_tensor("w3", [128, 2, 16], f8, kind="ExternalInput").ap()
    d_out = nc.dram_tensor("out", [NCH, DO, CH], f32, kind="ExternalOutput").ap()

    with tile.TileContext(nc) as tc:
        with (
            tc.tile_pool(name="wp", bufs=1) as wp,
            tc.tile_pool(name="xp", bufs=8) as xp,
            tc.tile_pool(name="ap_", bufs=2) as apool,
            tc.tile_pool(name="a2p", bufs=2) as a2pool,
            tc.tile_pool(name="op", bufs=2) as op,
            tc.tile_pool(name="ps1p", bufs=1, space="PSUM") as ps1p,
            tc.tile_pool(name="ps2p", bufs=1, space="PSUM") as ps2p,
            tc.tile_pool(name="pspk", bufs=2, space="PSUM") as pspk,
        ):
            w1h = wp.tile([128, KH, 2, H1], f8, name="w1h")
            w1l = wp.tile([128, KL + 1, H1], f16, name="w1l")
            w2a = wp.tile([128, 2, 256], f8, name="w2a")
            w2b = wp.tile([128, GRP, 2, 256], f8, name="w2b")
            w3 = wp.tile([128, 2, 16], f8, name="w3")
            nc.sync.dma_start(out=w1h[:], in_=d_w1h)

            # HAM/P-state pre-warm: dummy matmuls keep the PE busy during the
            # initial weight/x DMA wait so the first real matmuls run at full
            # clock (the activity window is ~3.4us).
            warm = wp.tile([128, 64], f16, name="warm")
            nc.vector.memset(warm[:], 1.0)
            wps = pspk.tile([64, 64], f32, name="wps", tag="pack")
            for _ in range(NWARM):
                nc.tensor.matmul(wps[:], warm[:, 0:64], warm[:], start=True, stop=True)

            # zero the a2 pair-1 garbage partitions once per rotating buffer
            # (w3 is also zero there; this guards against NaN/Inf x 0)
            for _ in range(2):
                a2z = a2pool.tile([128, 2, CH], f8, name="a2")
                nc.vector.memset(a2z[64:128, 1, :], 0.0)

            def layer1_mms(xhi, xlt, last=False, c0=0, c1=CH):
                """Layer-1 m-tile matmuls over batch columns [c0:c1]."""
                w = c1 - c0
                pss = []
                for m in range(3):
                    ps = ps1p.tile(
                        [128, CH], f32, name=f"ps1_{m}", bufs=(2 if m == 0 else 1)
                    )
                    for k in range(KH):
                        nc.tensor.matmul(
                            ps[:, 0:w],
                            w1h[:, k, :, m * 128 : (m + 1) * 128],
                            xhi[:, k, :, c0:c1],
                            start=(k == 0),
                            stop=False,
                            perf_mode=DR,
                        )
                    for k in range(KL):
                        nc.tensor.matmul(
                            ps[:, 0:w],
                            w1l[:, k, m * 128 : (m + 1) * 128],
                            xlt[:, k, c0:c1],
                            start=False,
                            stop=False,
                        )
                    if last:
                        # tail immediately per m-tile so Sign fires early and
                        # the final L2/L3 chain shortens the kernel epilogue
                        s = 32 * m
                        nc.tensor.matmul(
                            ps[:, 0:w],
                            w1l[s : s + 32, KL, m * 128 : (m + 1) * 128],
                            xlt[s : s + 32, KL, c0:c1],
                            start=False,
                            stop=True,
                        )
                    pss.append(ps)
                if not last:
                    # shared 32-row fp16 tail window: 3 m-tiles at partition
                    # strips 0/32/64 run concurrently in distinct row groups
                    for m in range(3):
                        s = 32 * m
                        nc.tensor.matmul(
                            pss[m][:, 0:w],
                            w1l[s : s + 32, KL, m * 128 : (m + 1) * 128],
                            xlt[s : s + 32, KL, c0:c1],
                            start=False,
                            stop=True,
                        )
                return pss

            def layer1_acts_early(pss, c0=0, c1=CH):
                """Urgent Sign activations: m1/m2 free single-buffered ps1
                banks that the next chunk's first DR matmuls reuse."""
                w = c1 - c0
                a1p = apool.tile([128, 2, CH], f8, name="a1p")
                a1x = apool.tile([128, 2, CH], f8, name="a1x")
                nc.scalar.activation(a1p[:, 1, 0:w], pss[1][:, 0:w], Sign)
                nc.scalar.activation(a1x[:, 0, 0:w], pss[2][:, 0:w], Sign)
                return a1p, a1x

            def layer1_act_m0(a1p, pss, c0=0, c1=CH):
                # ps1 m0 is double-buffered; its Sign can run last
                w = c1 - c0
                nc.scalar.activation(a1p[:, 0, 0:w], pss[0][:, 0:w], Sign)

            def m4_group(xhis, xlts):
                """Packed m4 (features 384:400) for 4 chunks: matmuls at col
                strips 0/32/64/96 run concurrently. Returns a13p (e4m3)."""
                ps4 = pspk.tile([128, CH], f32, name="ps4", tag="pack")
                nc.vector.memset(ps4[:], 0.0)
                for k in range(KH):
                    for i in range(2):
                        for jj in range(GRP):
                            s = 32 * jj
                            nc.tensor.matmul(
                                ps4[s : s + 16, :],
                                w1h[:, k, i, 384:400],
                                xhis[jj][:, k, i, :],
                                start=False,
                                stop=False,
                                tile_position=(0, s),
                            )
                for k in range(KL):
                    for jj in range(GRP):
                        s = 32 * jj
                        nc.tensor.matmul(
                            ps4[s : s + 16, :],
                            w1l[:, k, 384:400],
                            xlts[jj][:, k, :],
                            start=False,
                            stop=False,
                            tile_position=(0, s),
                        )
                for jj in range(GRP):
                    s = 32 * jj
                    nc.tensor.matmul(
                        ps4[s : s + 16, :],
                        w1l[0:32, KL, 384:400],
                        xlts[jj][0:32, KL, :],
                        start=False,
                        stop=(jj == GRP - 1),
                        tile_position=(0, s),
                    )
                a13p = apool.tile([128, CH], f8, name="a13p")
                nc.scalar.activation(a13p[:], ps4[:], Sign)
                return a13p

            def layer2_mms(jj, a1p, a1x, c0=0, c1=CH):
                """Layer 2: two DoubleRow matmuls per m-tile. DR#1s first so
                the DR#2s (which need the a13p copy) get extra stream cover."""
                w = c1 - c0
                pss2 = []
                for m, msz in ((0, 128), (1, 72)):
                    mo = m * 128
                    ps = ps2p.tile([msz, CH], f32, name=f"ps2_{m}")
                    nc.tensor.matmul(
                        ps[:, 0:w],
                        w2a[:, :, mo : mo + msz],
                        a1p[:, :, 0:w],
                        start=True,
                        stop=False,
                        perf_mode=DR,
                    )
                    pss2.append(ps)
                for m, msz in ((0, 128), (1, 72)):
                    mo = m * 128
                    nc.tensor.matmul(
                        pss2[m][:, 0:w],
                        w2b[:, jj, :, mo : mo + msz],
                        a1x[:, :, 0:w],
                        start=False,
                        stop=True,
                        perf_mode=DR,
                    )
                return pss2

            def layer2_acts(pss2, c0=0, c1=CH):
                w = c1 - c0
                a2 = a2pool.tile([128, 2, CH], f8, name="a2")
                nc.scalar.activation(a2[:, 0, 0:w], pss2[0][:, 0:w], Sign)
                nc.scalar.activation(a2[0:72, 1, 0:w], pss2[1][:, 0:w], Sign)
                return a2

            def layer3(c, a2, c0=0, c1=CH):
                w = c1 - c0
                ps3 = pspk.tile([128, CH], f32, name="ps3", tag="pack")
                nc.tensor.matmul(
                    ps3[0:DO, 0:w],
                    w3[:, :, 0:DO],
                    a2[:, :, 0:w],
                    start=True,
                    stop=True,
                    perf_mode=DR,
                )
                osb = op.tile([16, CH], f32, name="osb")
                nc.vector.tensor_copy(osb[0:DO, 0:w], ps3[0:DO, 0:w])
                nc.sync.dma_start(out=d_out[c, :, c0:c1], in_=osb[0:DO, 0:w])

            def dma_group(g):
                xhis, xlts = [], []
                for jj in range(GRP):
                    c = g * GRP + jj
                    xhi = xp.tile([128, KH, 2, CH], f8, name="xhi")
                    xlt = xp.tile([128, KL + 1, CH], f16, name="xlt")
                    if g == 0 and jj == 0:
                        # split the first chunk's transfers so the PE can
                        # start on partial data as it lands
                        nc.sync.dma_start(out=xhi[:, 0, :, :], in_=d_xhi[c, :, 0])
                        nc.sync.dma_start(out=xhi[:, 1:KH, :, :], in_=d_xhi[c, :, 1:KH])
                        nc.sync.dma_start(out=w1l[:, 0:3, :], in_=d_w1l[:, 0:3])
                        nc.sync.dma_start(out=xlt[:, 0:3, :], in_=d_xlt[c, :, 0:3])
                        nc.sync.dma_start(out=w1l[:, 3 : KL + 1, :], in_=d_w1l[:, 3 : KL + 1])
                        nc.sync.dma_start(out=xlt[:, 3 : KL + 1, :], in_=d_xlt[c, :, 3 : KL + 1])
                    else:
                        nc.sync.dma_start(out=xhi[:], in_=d_xhi[c])
                        nc.sync.dma_start(out=xlt[:], in_=d_xlt[c])
                    xhis.append(xhi)
                    xlts.append(xlt)
                    if g == 0 and jj == 1:
                        nc.sync.dma_start(out=w2a[:], in_=d_w2a)
                        nc.sync.dma_start(out=w2b[:], in_=d_w2b)
                        nc.sync.dma_start(out=w3[:], in_=d_w3)
                return xhis, xlts

            # -------- software-pipelined emission over the 16 chunks --------
            # Per steady step c (PE order): L1-matmuls(c), [m4-matmuls(G) on
            # group steps], L2-matmuls(c-1), L3(c-2). Scalar order within the
            # step: a2-acts(c-1) BEFORE a1-acts(c) so the ps2 banks free up
            # one chunk ahead of their reuse; a13p act right after m4 so the
            # DVE copies into a1x pair 1 are ready for L2 of the group.
            xs = {}
            ps1 = {}
            ps2 = {}
            a1 = {}
            a2 = {}
            a13 = {}

            def m4_mms(g):
                cs = [g * GRP + j for j in range(GRP)]
                return m4_group([xs[c][0] for c in cs], [xs[c][1] for c in cs])

            def a13_copy(c):
                if c // GRP in a13:
                    nc.vector.tensor_copy(a1[c][1][:, 1, :], a13[c // GRP][:])

            def prefetch(g):
                if g < NG:
                    xh, xl = dma_group(g)
                    for j in range(GRP):
                        xs[g * GRP + j] = (xh[j], xl[j])

            # preamble (DMA-bound): chunks 0 and 1, m4 of group 0
            prefetch(0)
            ps1[0] = layer1_mms(*xs[0])
            prefetch(1)
            a1[0] = layer1_acts_early(ps1[0])
            layer1_act_m0(a1[0][0], ps1[0])
            ps1[1] = layer1_mms(*xs[1])
            a13[0] = m4_mms(0)
            a1[1] = layer1_acts_early(ps1[1])
            layer1_act_m0(a1[1][0], ps1[1])
            a13_copy(0)
            a13_copy(1)
            ps2[0] = layer2_mms(0, *a1[0])
            a2[0] = layer2_acts(ps2.pop(0))
            for c in range(2, NCH - 1):
                g, jj = divmod(c, GRP)
                if jj == 1:
                    prefetch(g + 1)
                ps1[c] = layer1_mms(*xs[c])
                if jj == 0:
                    a13p_new = m4_mms(g)  # act emitted below, after a2 acts
                ps2[c - 1] = layer2_mms((c - 1) % GRP, *a1[c - 1])
                # scalar queue order: urgent a1 m1/m2 (ps1 bank reuse), then
                # a2 of c-1 (ps2 bank reuse next chunk), then a1 m0
                a1[c] = layer1_acts_early(ps1[c])
                a2[c - 1] = layer2_acts(ps2.pop(c - 1))
                layer1_act_m0(a1[c][0], ps1.pop(c))
                if jj == 0:
                    a13[g] = a13p_new
                a13_copy(c)
                layer3(c - 2, a2.pop(c - 2))
            # epilogue: chunk 15 is processed as two column halves with
            # per-m tails, so its Signs fire while L1 still streams and the
            # trailing act/L3 chain operates on half-width tiles
            cL = NCH - 1
            jL = cL % GRP
            HW_ = CH // 2

            def half(c0, c1):
                pssA = layer1_mms(*xs[cL], last=True, c0=c0, c1=c1)
                aA = layer1_acts_early(pssA, c0, c1)
                layer1_act_m0(aA[0], pssA, c0, c1)
                nc.vector.tensor_copy(aA[1][:, 1, 0 : c1 - c0], a13[cL // GRP][:, c0:c1])
                return aA

            aA = half(0, HW_)
            ps2[cL - 1] = layer2_mms((cL - 1) % GRP, *a1[cL - 1])
            a2[cL - 1] = layer2_acts(ps2.pop(cL - 1))
            layer3(cL - 2, a2.pop(cL - 2))
            aB = half(HW_, CH)
            psA = layer2_mms(jL, *aA, 0, HW_)
            a2A = layer2_acts(psA, 0, HW_)
            layer3(cL - 1, a2.pop(cL - 1))
            psB = layer2_mms(jL, *aB, HW_, CH)
            a2B = layer2_acts(psB, HW_, CH)
            layer3(cL, a2A, 0, HW_)
            layer3(cL, a2B, HW_, CH)

    nc.compile()
    _cache["nc"] = nc
    return nc


def _prep_weights(W1, W2, W3):
    s1 = np.sign(W1).T.astype(np.float32)  # [784, 400]
    w1h = np.ascontiguousarray(
        s1[:768].reshape(KH, 2, 128, H1).transpose(2, 0, 1, 3)
    ).astype(E4M3)  # [128, 3, 2, 400]
    w1l = np.zeros((128, KL + 1, H1), np.float16)
    w1l[:, 0:KL, :] = s1[:768].reshape(KL, 128, H1).transpose(1, 0, 2)
    trip = np.concatenate([s1[768:784], s1[768:784]], axis=0)  # [32, 400]
    for m in range(3):
        w1l[32 * m : 32 * m + 32, KL, :] = trip

    s2 = np.sign(W2).T.astype(np.float32)  # [400, 200]
    w2a = np.zeros((128, 2, 256), np.float32)
    w2a[:, 0, 0:H2] = s2[0:128]
    w2a[:, 1, 0:H2] = s2[128:256]
    w2b = np.zeros((128, GRP, 2, 256), np.float32)
    for jj in range(GRP):
        w2b[:, jj, 0, 0:H2] = s2[256:384]
        w2b[32 * jj : 32 * jj + 16, jj, 1, 0:H2] = s2[384:400]

    s3 = np.sign(W3).T.astype(np.float32)  # [200, 10]
    w3 = np.zeros((128, 2, 16), np.float32)
    w3[:, 0, 0:DO] = s3[0:128]
    w3[0:72, 1, 0:DO] = s3[128:200]

    return w1h, w1l, w2a.astype(E4M3), w2b.astype(E4M3), w3.astype(E4M3)


def _prep_x_core(xc):
    # xc: [8192, 784] fp32 -> hi e4m3 [16, 128, 3, 2, 512], lo+tail fp16
    # [16, 128, 7, 512]
    xt = np.ascontiguousarray(xc.T.astype(np.float32))  # [784, 8192]
    hi8 = xt.astype(E4M3)
    lo = (xt - hi8.astype(np.float32)).astype(np.float16)  # [784, 8192]
    xhi = np.ascontiguousarray(
        hi8[:768].reshape(KH, 2, 128, NCH, CH).transpose(3, 2, 0, 1, 4)
    )  # [16, 128, 3, 2, 512]
    xlt = np.zeros((NCH, 128, KL + 1, CH), np.float16)
    xlt[:, :, 0:KL, :] = lo[:768].reshape(KL, 128, NCH, CH).transpose(2, 1, 0, 3)
    hi16 = hi8[768:784].astype(np.float16)  # exact
    tail = np.zeros((128, BL), np.float16)
    for m in range(3):
        tail[32 * m : 32 * m + 16] = hi16
        tail[32 * m + 16 : 32 * m + 32] = lo[768:784]
    xlt[:, :, KL, :] = tail.reshape(128, NCH, CH).transpose(1, 0, 2)
    return xhi, np.ascontiguousarray(xlt)


def kernel(x, W1, W2, W3, _trace=False, **_kw):
    nc = _build()
    w1h, w1l, w2a, w2b, w3 = _prep_weights(
        np.asarray(W1, np.float32), np.asarray(W2, np.float32), np.asarray(W3, np.float32)
    )
    x = np.asarray(x, np.float32).reshape(B, D0)

    in_maps = []
    for c in range(NCORES):
        xhi, xlt = _prep_x_core(x[c * BL : (c + 1) * BL])
        in_maps.append(
            {
                "xhi": xhi,
                "xlt": xlt,
                "w1h": w1h,
                "w1l": w1l,
                "w2a": w2a,
                "w2b": w2b,
                "w3": w3,
            }
        )

    _ensure_axon_hooks()
    res = run_bass_kernel_spmd(nc, in_maps, core_ids=list(range(NCORES)), trace=_trace)

    out = np.empty((B, DO), np.float32)
    for c in range(NCORES):
        oc = res.results[c]["out"]  # [16, 10, 512]
        out[c * BL : (c + 1) * BL] = oc.transpose(0, 2, 1).reshape(BL, DO)
    if _trace:
        _cache["last_results"] = res
    return out
